# revision 18
# baseline (speedup 1.0000x reference)
"""Cdist-mean kernel for Trainium2 (8 NeuronCores, SPMD row-sharded).

Computes mean(cdist(x.reshape(T,-1), y.reshape(T,-1))) for T=8192, D=512.

Strategy per core c (of 8): rows x[c*1024:(c+1)*1024] vs all of y.
  sq[i,j] = x2[i] + y2[j] - 2*x.y  via bf16 matmul with K on partitions:
    - 4 matmuls (K=128 chunks) accumulate x.y into PSUM
    - 1 augmented K=1 matmul adds -y2[j]/2 (rhs row precomputed on device)
    - ACT: sqrt(-2*psum + x2[i])  (per-partition bias), with accum_out
      doing the free-dim sum reduction in the same instruction.
  Per-core result: [128, 128] partial sums; host sums and divides by T^2.
"""

import sys

import numpy as np

if "/opt/trn_rl_repo" not in sys.path:
    sys.path.insert(0, "/opt/trn_rl_repo")

import ml_dtypes

T = 8192
D = 512  # flattened feature dim (256*2)
NCORES = 8
M = T // NCORES  # 1024 rows of x per core
P = 128
KC = D // P  # 4 K-chunks
MT = M // P  # 8 m-tiles per core
SEG = 512  # n-segment (matmul free dim)
NSEG = T // SEG  # 16

_CACHE = {}


def _build():
    import concourse.bass as bass
    import concourse.tile as tile
    from concourse import bacc, mybir

    nc = bacc.Bacc(
        "TRN2",
        target_bir_lowering=False,
        debug=False,
        enable_asserts=False,
        num_devices=NCORES,
    )

    xs = nc.dram_tensor("xs", [M, D], mybir.dt.bfloat16, kind="ExternalInput").ap()
    yb = nc.dram_tensor("yb", [T, D], mybir.dt.bfloat16, kind="ExternalInput").ap()
    out = nc.dram_tensor(
        "out", [P, 48], mybir.dt.float32, kind="ExternalOutput"
    ).ap()

    with tile.TileContext(nc) as tc:
        with (
            tc.tile_pool(name="persist", bufs=1) as persist,
            tc.tile_pool(name="work", bufs=3) as work,
            tc.tile_pool(name="psum", bufs=2, space="PSUM") as pp,
            tc.tile_pool(name="psum_y2", bufs=1, space="PSUM") as pp_y2,
        ):
            f32 = mybir.dt.float32
            bf16 = mybir.dt.bfloat16

            # ---- persistent tiles ----
            yt = [persist.tile([P, T], bf16, tag=f"yt{kc}", name=f"yt{kc}") for kc in range(KC)]
            xt = [persist.tile([P, M], bf16, tag=f"xt{kc}", name=f"xt{kc}") for kc in range(KC)]
            # aug rhs, K padded to 128 so its LDWEIGHTS hides like the main
            # matmuls': row0 = ones, row1 = -y2[j]/2, rows 2..127 = 0
            aug = persist.tile([P, T], bf16, tag="aug")
            # aug lhsT: row0 = -x2[m]/2, row1 = ones, rows 2..127 = 0
            augL = persist.tile([P, M], bf16, tag="augL")
            acc_cols = persist.tile([P, 48], f32, tag="acc_cols")
            ones_col2 = persist.tile([P, 2], bf16, tag="ones_col2")
            # per-partition scale/bias for the y2 ACT: row0 = 0*in+1 = 1.0,
            # row1 = -0.5*in + 0 = -y2/2
            sc_y2 = persist.tile([2, 1], f32, tag="sc_y2")
            bi_y2 = persist.tile([2, 1], f32, tag="bi_y2")

            nc.vector.memset(ones_col2[:], 1.0)
            nc.gpsimd.memset(aug[:], 0.0)
            nc.vector.memset(augL[:], 0.0)
            nc.vector.memset(augL[0:2, :], 1.0)
            nc.vector.memset(sc_y2[:], -0.5)
            nc.vector.memset(sc_y2[0:1, :], 0.0)
            nc.vector.memset(bi_y2[:], 0.0)
            nc.vector.memset(bi_y2[0:1, :], 1.0)

            f8 = mybir.dt.float8e4
            # fp8 copies of the transposed operands for DoubleRow matmuls
            yt8 = persist.tile([P, KC, T], f8, tag="yt8")
            xt8 = persist.tile([P, KC, M], f8, tag="xt8")

            # ---- transposes: xt on the scalar HWDGE queue, y on sync, so
            # the two streams overlap and the first main group starts early
            # xt[kc][k, m] = x[m, kc*128+k]
            for kc in range(KC):
                nc.scalar.dma_start_transpose(
                    xt[kc][:], xs[:, kc * P : (kc + 1) * P]
                )
            for kc in range(KC):
                nc.vector.tensor_copy(xt8[:, kc, :], xt[kc][:])
            y_chunks = [(0, 1024), (1024, 1024), (2048, 1536), (3584, 1536), (5120, 1536), (6656, 1536)]
            for q0, qw in y_chunks:
                for kc in range(KC):
                    nc.sync.dma_start_transpose(
                        yt[kc][:, q0 : q0 + qw],
                        yb[q0 : q0 + qw, kc * P : (kc + 1) * P],
                    )

            # ---- x2 row: augL[0, m] = -x2[m]/2 via ones-matmul over xt^2
            # (issued after the first y2_preps so the prologue DVE FIFO
            # prioritizes what the first main matmuls need) ----
            def x2_prep():
                for h in range(M // SEG):
                    ps_x2 = pp_y2.tile([2, SEG], f32, tag="ps_y2", name="ps_x2")
                    for kc in range(KC):
                        xsq = work.tile([P, SEG], bf16, tag="ysq", name="xsq")
                        seg = xt[kc][:, h * SEG : (h + 1) * SEG]
                        nc.vector.tensor_tensor(
                            xsq[:], seg, seg, mybir.AluOpType.mult
                        )
                        nc.tensor.matmul(
                            ps_x2[0:1, :],
                            ones_col2[:, 0:1],
                            xsq[:],
                            start=(kc == 0),
                            stop=(kc == KC - 1),
                        )
                    nc.scalar.activation(
                        augL[0:1, h * SEG : (h + 1) * SEG],
                        ps_x2[0:1, :],
                        mybir.ActivationFunctionType.Copy,
                        scale=-0.5,
                    )

            # y2 prep for one segment: aug[0, j] = -y2[j]/2 (bf16).
            # Issued just-in-time inside the main loop so a y2 matmul for a
            # not-yet-DMA'd segment never blocks resident main matmuls in
            # the PE's FIFO queue.
            def y2_prep(s):
                ps_y2 = pp_y2.tile([2, SEG], f32, tag="ps_y2", name="ps_y2")
                for kc in range(KC):
                    seg = yt[kc][:, s * SEG : (s + 1) * SEG]
                    # fp8 copy for the DoubleRow mains, cast just-in-time so
                    # the DVE FIFO never blocks on a not-yet-DMA'd chunk
                    nc.vector.tensor_copy(yt8[:, kc, s * SEG : (s + 1) * SEG], seg)
                    ysq = work.tile([P, SEG], bf16, tag="ysq", name="ysq")
                    nc.vector.tensor_tensor(
                        ysq[:], seg, seg, mybir.AluOpType.mult
                    )
                    nc.tensor.matmul(
                        ps_y2[:],
                        ones_col2[:],
                        ysq[:],
                        start=(kc == 0),
                        stop=(kc == KC - 1),
                    )
                # per-partition scale/bias on DVE (keeps ACT free for sqrt):
                # row0 = 0*in + 1 = 1.0 exactly, row1 = -0.5*in + 0 = -y2/2
                nc.vector.tensor_scalar(
                    aug[0:2, s * SEG : (s + 1) * SEG],
                    ps_y2[:],
                    sc_y2[:],
                    bi_y2[:],
                    mybir.AluOpType.mult,
                    mybir.AluOpType.add,
                )

            # ---- main loop: several segments share one multi-bank PSUM
            # tile so a single ACT sqrt (+accum) covers them all ----
            GROUPS = [2, 2, 3, 3, 3, 3]  # seg counts; 3 banks x 2 bufs + 2 = 8
            GMAX = max(GROUPS)
            col = 0
            s0 = 0
            for nb, gn in enumerate(GROUPS):
                for g in range(gn):
                    y2_prep(s0 + g)
                if nb == 0:
                    x2_prep()
                for mi in range(MT):
                    psum = pp.tile([P, GMAX * SEG], f32, tag="psum", name="psum")
                    for g in range(gn):
                        ni = s0 + g
                        sub = psum[:, g * SEG : (g + 1) * SEG]
                        for c2 in range(KC // 2):
                            nc.tensor.matmul(
                                sub,
                                xt8[:, 2 * c2 : 2 * c2 + 2, mi * P : (mi + 1) * P],
                                yt8[:, 2 * c2 : 2 * c2 + 2, ni * SEG : (ni + 1) * SEG],
                                start=(c2 == 0),
                                stop=False,
                                perf_mode=mybir.MatmulPerfMode.DoubleRow,
                            )
                        nc.tensor.matmul(
                            sub,
                            augL[:, mi * P : (mi + 1) * P],
                            aug[:, ni * SEG : (ni + 1) * SEG],
                            start=False,
                            stop=True,
                        )
                    nc.scalar.activation(
                        psum[:, : gn * SEG],
                        psum[:, : gn * SEG],
                        mybir.ActivationFunctionType.Sqrt,
                        scale=-2.0,
                        accum_out=acc_cols[:, col : col + 1],
                    )
                    col += 1
                s0 += gn

            nc.sync.dma_start(out[:], acc_cols[:])

    nc.compile()
    return nc


def _get_nc():
    if "nc" not in _CACHE:
        _CACHE["nc"] = _build()
    return _CACHE["nc"]


def _run(x, y, trace=False, **kw):
    from concourse.bass_utils import run_bass_kernel_spmd

    xf = np.ascontiguousarray(np.asarray(x, dtype=np.float32).reshape(T, D))
    yf = np.ascontiguousarray(np.asarray(y, dtype=np.float32).reshape(T, D))
    xb = xf.astype(ml_dtypes.bfloat16)
    ybv = yf.astype(ml_dtypes.bfloat16)

    nc = _get_nc()
    in_maps = [
        {"xs": np.ascontiguousarray(xb[c * M : (c + 1) * M]), "yb": ybv}
        for c in range(NCORES)
    ]
    res = run_bass_kernel_spmd(
        nc, in_maps, core_ids=list(range(NCORES)), trace=trace, **kw
    )
    total = sum(float(r["out"].astype(np.float64).sum()) for r in res.results)
    val = np.float32(total / (float(T) * float(T)))
    return np.array(val, dtype=np.float32), res


def kernel(x, y):
    out, _ = _run(x, y)
    return out


# revision 19
# speedup vs baseline: 1.0317x; 1.0317x over previous
"""Cdist-mean kernel for Trainium2 (8 NeuronCores, SPMD row-sharded).

Computes mean(cdist(x.reshape(T,-1), y.reshape(T,-1))) for T=8192, D=512.

Strategy per core c (of 8): rows x[c*1024:(c+1)*1024] vs all of y.
  sq[i,j] = x2[i] + y2[j] - 2*x.y  via bf16 matmul with K on partitions:
    - 4 matmuls (K=128 chunks) accumulate x.y into PSUM
    - 1 augmented K=1 matmul adds -y2[j]/2 (rhs row precomputed on device)
    - ACT: sqrt(-2*psum + x2[i])  (per-partition bias), with accum_out
      doing the free-dim sum reduction in the same instruction.
  Per-core result: [128, 128] partial sums; host sums and divides by T^2.
"""

import sys

import numpy as np

if "/opt/trn_rl_repo" not in sys.path:
    sys.path.insert(0, "/opt/trn_rl_repo")

import ml_dtypes

T = 8192
D = 512  # flattened feature dim (256*2)
NCORES = 8
M = T // NCORES  # 1024 rows of x per core
P = 128
KC = D // P  # 4 K-chunks
MT = M // P  # 8 m-tiles per core
SEG = 512  # n-segment (matmul free dim)
NSEG = T // SEG  # 16

_CACHE = {}


def _build():
    import concourse.bass as bass
    import concourse.tile as tile
    from concourse import bacc, mybir

    nc = bacc.Bacc(
        "TRN2",
        target_bir_lowering=False,
        debug=False,
        enable_asserts=False,
        num_devices=NCORES,
    )

    xs = nc.dram_tensor("xs", [M, D], mybir.dt.bfloat16, kind="ExternalInput").ap()
    yb = nc.dram_tensor("yb", [T, D], mybir.dt.bfloat16, kind="ExternalInput").ap()
    out = nc.dram_tensor(
        "out", [P, 48], mybir.dt.float32, kind="ExternalOutput"
    ).ap()

    with tile.TileContext(nc) as tc:
        with (
            tc.tile_pool(name="persist", bufs=1) as persist,
            tc.tile_pool(name="work", bufs=3) as work,
            tc.tile_pool(name="psum", bufs=2, space="PSUM") as pp,
            tc.tile_pool(name="psum_y2", bufs=2, space="PSUM") as pp_y2,
        ):
            f32 = mybir.dt.float32
            bf16 = mybir.dt.bfloat16

            # ---- persistent tiles ----
            yt = [persist.tile([P, T], bf16, tag=f"yt{kc}", name=f"yt{kc}") for kc in range(KC)]
            xt = [persist.tile([P, M], bf16, tag=f"xt{kc}", name=f"xt{kc}") for kc in range(KC)]
            # aug rhs, K padded to 128 so its LDWEIGHTS hides like the main
            # matmuls': row0 = ones, row1 = -y2[j]/2, rows 2..127 = 0
            aug = persist.tile([P, T], bf16, tag="aug")
            # aug lhsT: row0 = -x2[m]/2, row1 = ones, rows 2..127 = 0
            augL = persist.tile([P, M], bf16, tag="augL")
            acc_cols = persist.tile([P, 48], f32, tag="acc_cols")
            ones_col2 = persist.tile([P, 2], bf16, tag="ones_col2")
            # per-partition scale/bias for the y2 ACT: row0 = 0*in+1 = 1.0,
            # row1 = -0.5*in + 0 = -y2/2
            sc_y2 = persist.tile([2, 1], f32, tag="sc_y2")
            bi_y2 = persist.tile([2, 1], f32, tag="bi_y2")

            nc.vector.memset(ones_col2[:], 1.0)
            nc.gpsimd.memset(aug[:], 0.0)
            nc.vector.memset(augL[:], 0.0)
            nc.vector.memset(augL[0:2, :], 1.0)
            nc.vector.memset(sc_y2[:], -0.5)
            nc.vector.memset(sc_y2[0:1, :], 0.0)
            nc.vector.memset(bi_y2[:], 0.0)
            nc.vector.memset(bi_y2[0:1, :], 1.0)

            f8 = mybir.dt.float8e4
            # fp8 copies of the transposed operands for DoubleRow matmuls
            yt8 = persist.tile([P, KC, T], f8, tag="yt8")
            xt8 = persist.tile([P, KC, M], f8, tag="xt8")

            # ---- transposes: xt on the scalar HWDGE queue, y on sync, so
            # the two streams overlap and the first main group starts early
            # xt[kc][k, m] = x[m, kc*128+k]
            for kc in range(KC):
                nc.scalar.dma_start_transpose(
                    xt[kc][:], xs[:, kc * P : (kc + 1) * P]
                )
            for kc in range(KC):
                nc.vector.tensor_copy(xt8[:, kc, :], xt[kc][:])
            y_chunks = [(0, 1024), (1024, 1024), (2048, 1536), (3584, 1536), (5120, 1536), (6656, 1536)]
            for q0, qw in y_chunks:
                for kc in range(KC):
                    nc.sync.dma_start_transpose(
                        yt[kc][:, q0 : q0 + qw],
                        yb[q0 : q0 + qw, kc * P : (kc + 1) * P],
                    )

            # ---- x2 row: augL[0, m] = -x2[m]/2 via ones-matmul over xt^2
            # (issued after the first y2_preps so the prologue DVE FIFO
            # prioritizes what the first main matmuls need) ----
            def x2_prep():
                for h in range(M // SEG):
                    ps_x2 = pp_y2.tile([2, SEG], f32, tag="ps_y2", name="ps_x2")
                    for kc in range(KC):
                        xsq = work.tile([P, SEG], bf16, tag="ysq", name="xsq")
                        seg = xt[kc][:, h * SEG : (h + 1) * SEG]
                        nc.vector.tensor_tensor(
                            xsq[:], seg, seg, mybir.AluOpType.mult
                        )
                        nc.tensor.matmul(
                            ps_x2[0:1, :],
                            ones_col2[:, 0:1],
                            xsq[:],
                            start=(kc == 0),
                            stop=(kc == KC - 1),
                        )
                    nc.scalar.activation(
                        augL[0:1, h * SEG : (h + 1) * SEG],
                        ps_x2[0:1, :],
                        mybir.ActivationFunctionType.Copy,
                        scale=-0.5,
                    )

            # y2 prep for one segment: aug[0, j] = -y2[j]/2 (bf16).
            # Issued just-in-time inside the main loop so a y2 matmul for a
            # not-yet-DMA'd segment never blocks resident main matmuls in
            # the PE's FIFO queue.
            def y2_prep(s):
                ps_y2 = pp_y2.tile([2, SEG], f32, tag="ps_y2", name="ps_y2")
                for kc in range(KC):
                    seg = yt[kc][:, s * SEG : (s + 1) * SEG]
                    # fp8 copy for the DoubleRow mains, cast just-in-time so
                    # the DVE FIFO never blocks on a not-yet-DMA'd chunk
                    nc.vector.tensor_copy(yt8[:, kc, s * SEG : (s + 1) * SEG], seg)
                    ysq = work.tile([P, SEG], bf16, tag="ysq", name="ysq")
                    nc.vector.tensor_tensor(
                        ysq[:], seg, seg, mybir.AluOpType.mult
                    )
                    nc.tensor.matmul(
                        ps_y2[:],
                        ones_col2[:],
                        ysq[:],
                        start=(kc == 0),
                        stop=(kc == KC - 1),
                    )
                # per-partition scale/bias on DVE (keeps ACT free for sqrt):
                # row0 = 0*in + 1 = 1.0 exactly, row1 = -0.5*in + 0 = -y2/2
                nc.vector.tensor_scalar(
                    aug[0:2, s * SEG : (s + 1) * SEG],
                    ps_y2[:],
                    sc_y2[:],
                    bi_y2[:],
                    mybir.AluOpType.mult,
                    mybir.AluOpType.add,
                )

            # ---- main loop: several segments share one multi-bank PSUM
            # tile so a single ACT sqrt (+accum) covers them all ----
            GROUPS = [2, 2, 3, 3, 3, 3]  # seg counts; 3 banks x 2 bufs + 2 = 8
            GMAX = max(GROUPS)
            col = 0
            s0 = 0
            for nb, gn in enumerate(GROUPS):
                for g in range(gn):
                    y2_prep(s0 + g)
                if nb == 0:
                    x2_prep()
                for mi in range(MT):
                    psum = pp.tile([P, GMAX * SEG], f32, tag="psum", name="psum")
                    for g in range(gn):
                        ni = s0 + g
                        sub = psum[:, g * SEG : (g + 1) * SEG]
                        for c2 in range(KC // 2):
                            nc.tensor.matmul(
                                sub,
                                xt8[:, 2 * c2 : 2 * c2 + 2, mi * P : (mi + 1) * P],
                                yt8[:, 2 * c2 : 2 * c2 + 2, ni * SEG : (ni + 1) * SEG],
                                start=(c2 == 0),
                                stop=False,
                                perf_mode=mybir.MatmulPerfMode.DoubleRow,
                            )
                        nc.tensor.matmul(
                            sub,
                            augL[:, mi * P : (mi + 1) * P],
                            aug[:, ni * SEG : (ni + 1) * SEG],
                            start=False,
                            stop=True,
                        )
                    nc.scalar.activation(
                        psum[:, : gn * SEG],
                        psum[:, : gn * SEG],
                        mybir.ActivationFunctionType.Sqrt,
                        scale=-2.0,
                        accum_out=acc_cols[:, col : col + 1],
                    )
                    col += 1
                s0 += gn

            nc.sync.dma_start(out[:], acc_cols[:])

    nc.compile()
    return nc


def _get_nc():
    if "nc" not in _CACHE:
        _CACHE["nc"] = _build()
    return _CACHE["nc"]


def _run(x, y, trace=False, **kw):
    from concourse.bass_utils import run_bass_kernel_spmd

    xf = np.ascontiguousarray(np.asarray(x, dtype=np.float32).reshape(T, D))
    yf = np.ascontiguousarray(np.asarray(y, dtype=np.float32).reshape(T, D))
    xb = xf.astype(ml_dtypes.bfloat16)
    ybv = yf.astype(ml_dtypes.bfloat16)

    nc = _get_nc()
    in_maps = [
        {"xs": np.ascontiguousarray(xb[c * M : (c + 1) * M]), "yb": ybv}
        for c in range(NCORES)
    ]
    res = run_bass_kernel_spmd(
        nc, in_maps, core_ids=list(range(NCORES)), trace=trace, **kw
    )
    total = sum(float(r["out"].astype(np.float64).sum()) for r in res.results)
    val = np.float32(total / (float(T) * float(T)))
    return np.array(val, dtype=np.float32), res


def kernel(x, y):
    out, _ = _run(x, y)
    return out
